# revision 23
# baseline (speedup 1.0000x reference)
"""GCN layer (normalized adjacency aggregation) on 8 Trainium2 NeuronCores.

Algorithm (row-sharded, bf16, fully SBUF-resident adjacency):
    a_hat = A + I  (identity folded into the shard on the host, then bf16)
    deg[i] = sum_j a_hat[i, j]              -> per-core PE ones-matmul over its rows
    AllGather raw degrees (4KB), rsqrt computed on both sides of the gather
    sup = x @ W.T + b  (bf16)               -> computed redundantly per core (tiny)
    S = dinv[:, None] * sup
    out[i, :] = dinv[i] * (a_hat[i, :] @ S) -> accumulating matmul over the row block

Performance structure:
  - the transposed adjacency shard is pre-tiled on the HOST into the exact SBUF
    tile layout (one contiguous run per partition), so tile DMAs need no
    software descriptor generation.  14 full tiles (4 j-blocks) + 8 mini tiles
    (1 j-block) ride three balanced DMA queues; x.T is split in halves across
    two queues; HBM is read exactly once and everything stays SBUF-resident.
  - the degree matmuls are emitted in estimated-arrival order of the tiles so
    the PE never head-of-line blocks on a late queue; mini tiles at the queue
    tails keep the post-stream backlog to ~2us.
  - support matmuls run early with a deep (6-buffer) PSUM pool so the trailing
    DVE bias-adds never stall the PE.
  - the AllGather moves raw degrees (PSUM copied out on DVE+Act halves); the
    gathered [64,128] view is transposed with one PE matmul into
    partition-major [128,64] and rsqrt'd there.
  - all 64 support-column scales run up front on DVE+Act, then the 128
    aggregation matmuls issue back-to-back; the final row-scale and output DMA
    are split in halves across two queues.
  - a tiny dummy collective keyed off a mid-stream tile DMA pre-wakes the CC
    cores so the real AllGather does not pay their ~11us wake latency.
"""

import numpy as np
from contextlib import ExitStack

N = 8192
F = 128
NCORES = 8
RPC = N // NCORES  # 1024 rows per core
P = 128            # partitions
JT = N // P        # 64 column tiles of the (transposed) block

NFULL = 12         # full tiles of FJ j-blocks each
FJ = 4
NMINI = JT - NFULL * FJ  # 16 mini tiles of 1 j-block

PREWAKE = True

# queue plans: lists of ("xt", half) | ("full", idx) | ("mini", idx) | ("consts",)
# gpsimd carries NO bulk DMA: its ring stays empty so the collective triggers
# (which only gpsimd can issue) are never blocked behind queued transfers.
QPLAN = {
    "sync":   [("xt", 0), ("full", 0), ("full", 2), ("full", 4), ("full", 6),
               ("full", 8), ("full", 10), ("mini", 0), ("mini", 2), ("mini", 4),
               ("mini", 6), ("mini", 8), ("mini", 10), ("mini", 12), ("mini", 14)],
    "gpsimd": [],
    "scalar": [("consts",), ("xt", 1), ("full", 1), ("full", 3), ("full", 5),
               ("full", 7), ("full", 9), ("full", 11), ("mini", 1), ("mini", 3),
               ("mini", 5), ("mini", 7), ("mini", 9), ("mini", 11),
               ("mini", 13), ("mini", 15)],
}
WAKE_AFTER = ("full", 5)  # dummy collective keyed off this tile (~33us arrival)


def _arrival_plan():
    """Estimated arrival times (us) per tile -> degree emission order."""
    rate = 0.15  # GB/s per queue in MB/us (two bulk queues sharing HBM)
    size = {"xt": 1.0, "full": 1.0, "mini": 0.25, "consts": 0.15}
    arrivals = {}
    for qname, items in QPLAN.items():
        t = 0.0
        for it in items:
            t += size[it[0]] / rate
            if it[0] in ("full", "mini"):
                arrivals[it] = t
    order = sorted(arrivals, key=lambda it: arrivals[it])
    return order


def build_module(n=N, f=F, ncores=NCORES, prewake=PREWAKE):
    """Build and compile the SPMD Bass module (same program on every core)."""
    from concourse import bass, bacc, tile

    mybir = bass.mybir
    dt = mybir.dt.float32
    bf = mybir.dt.bfloat16

    rpc = n // ncores
    jt = n // P

    nc = bacc.Bacc(
        "TRN2",
        target_bir_lowering=False,
        debug=False,
        enable_asserts=False,
        num_devices=ncores,
    )

    # att holds the pre-tiled shard: 14 full tiles then 8 mini tiles,
    # each [P, nj*rpc] with one contiguous run per partition.
    att_d = nc.dram_tensor("att", [(NFULL * FJ + NMINI) * P, rpc], bf,
                           kind="ExternalInput")
    xt_d = nc.dram_tensor("xt", [f, n], bf, kind="ExternalInput")
    wt_d = nc.dram_tensor("wt", [f, f], bf, kind="ExternalInput")
    b_d = nc.dram_tensor("bias", [1, f], dt, kind="ExternalInput")
    ones_r_d = nc.dram_tensor("ones_r", [1, P], dt, kind="ExternalInput")
    ones_c_d = nc.dram_tensor("ones_c", [P, 1], bf, kind="ExternalInput")
    id64_d = nc.dram_tensor("id64", [64, 64], dt, kind="ExternalInput")
    out_d = nc.dram_tensor("out_t", [f, rpc], dt, kind="ExternalOutput")

    def att_rows(it):
        """DRAM row range of tile `it` in att_d (viewed [P, nj*rpc])."""
        kind, idx = it
        if kind == "full":
            r0 = idx * FJ * P
            return r0, FJ
        r0 = (NFULL * FJ + idx) * P
        return r0, 1

    with tile.TileContext(nc) as tc, ExitStack() as ctx:
        cpool = ctx.enter_context(tc.tile_pool(name="const", bufs=1))
        wt_sb = cpool.tile([f, f], bf, name="wt_sb")
        bias_sb = cpool.tile([1, f], dt, name="bias_sb")
        ones_r = cpool.tile([1, P], dt, name="ones_r")
        ones_c = cpool.tile([P, 1], bf, name="ones_c")
        id64 = cpool.tile([64, 64], dt, name="id64")
        b_rep = cpool.tile([P, f], dt, name="b_rep")
        xt_sb = cpool.tile([f, n], bf, name="xt_sb")
        sup_bf = cpool.tile([P, jt * f], bf, name="sup_bf")
        deg_l = cpool.tile([1, rpc], dt, name="deg_l")
        deg_rep = cpool.tile([P, rpc], dt, name="deg_rep")
        sq_rep = cpool.tile([P, rpc], dt, name="sq_rep")
        rec_sb = cpool.tile([P, jt], dt, name="rec_sb")
        dinv_sb = cpool.tile([P, jt], dt, name="dinv_sb")
        dinv_rep = cpool.tile([P, rpc], dt, name="dinv_rep")
        ag_sb = cpool.tile([jt, P], dt, name="ag_sb")
        out_sb = cpool.tile([P, rpc], dt, name="out_sb")

        dram = ctx.enter_context(tc.tile_pool(name="dram", bufs=1, space="DRAM"))
        ag_in = dram.tile([1, rpc], dt, name="ag_in")
        ag_out = dram.tile([ncores, rpc], dt, name="ag_out",
                           addr_space="Shared" if ncores > 4 else "Local")
        if prewake:
            wk_in = dram.tile([1, 2], bf, name="wk_in")
            wk_out = dram.tile([ncores, 2], bf, name="wk_out",
                               addr_space="Shared" if ncores > 4 else "Local")

        afull = ctx.enter_context(tc.tile_pool(name="af", bufs=NFULL))
        amini = ctx.enter_context(tc.tile_pool(name="am", bufs=NMINI))
        a_tiles = {}
        engs = {"sync": nc.sync, "gpsimd": nc.gpsimd, "scalar": nc.scalar}
        for qname, items in QPLAN.items():
            eng = engs[qname]
            for it in items:
                if it[0] == "consts":
                    eng.dma_start(wt_sb[:], wt_d[:])
                    eng.dma_start(bias_sb[:], b_d[:])
                    eng.dma_start(ones_r[:], ones_r_d[:])
                    eng.dma_start(ones_c[:], ones_c_d[:])
                    eng.dma_start(id64[:], id64_d[:])
                elif it[0] == "xt":
                    h = it[1]
                    eng.dma_start(xt_sb[:, h * (n // 2):(h + 1) * (n // 2)],
                                  xt_d[:, h * (n // 2):(h + 1) * (n // 2)])
                else:
                    r0, nj = att_rows(it)
                    pool, tag = (afull, "af") if it[0] == "full" else (amini, "am")
                    t = pool.tile([P, nj * rpc], bf,
                                  name=f"a_{it[0]}{it[1]}", tag=tag)
                    eng.dma_start(t[:], att_d[r0:r0 + nj * P, :].rearrange(
                        "(p h) i -> p (h i)", p=P))
                    a_tiles[it] = t
                if prewake and it == WAKE_AFTER:
                    eng.dma_start(wk_in[:], a_tiles[it][0:1, 0:2])

        def tile_js(it):
            """Global j indices covered by tile `it`."""
            kind, idx = it
            if kind == "full":
                return [idx * FJ + h for h in range(FJ)]
            return [NFULL * FJ + idx]

        # ---- Phase A: support = x @ W.T + b, and degree row sums ----
        with (
            tc.tile_pool(name="psum_s", bufs=5, space="PSUM") as psum_s,
            tc.tile_pool(name="psum_b", bufs=1, space="PSUM") as psum_b,
            tc.tile_pool(name="psum_r", bufs=1, space="PSUM") as psum_r,
        ):
            # bias broadcast via outer product: ones_r.T @ bias -> [P, f]
            pb = psum_b.tile([P, f], dt, name="pb")
            nc.tensor.matmul(pb[:], ones_r[:], bias_sb[:], start=True, stop=True)
            nc.vector.tensor_copy(b_rep[:], pb[:])

            for j in range(jt):
                ps = psum_s.tile([P, f], dt, name=f"ps{j}", tag="ps")
                nc.tensor.matmul(ps[:], xt_sb[:, j * f:(j + 1) * f], wt_sb[:],
                                 start=True, stop=True)
                nc.vector.tensor_add(sup_bf[:, j * f:(j + 1) * f], ps[:], b_rep[:])

            if prewake:
                nc.gpsimd.collective_compute(
                    "AllGather",
                    mybir.AluOpType.bypass,
                    replica_groups=[list(range(ncores))],
                    ins=[wk_in.opt()],
                    outs=[wk_out.opt()],
                )

            # degree: accumulate ones_c.T @ a_tile into [1, rpc], tiles in
            # estimated-arrival order (accumulation is commutative)
            order = _arrival_plan()
            nmm = sum(len(tile_js(it)) for it in order) * 2
            pr = psum_r.tile([1, rpc], dt, name="pr")
            k = 0
            for it in order:
                t = a_tiles[it]
                for h in range(len(tile_js(it))):
                    for c in range(0, rpc, 512):
                        nc.tensor.matmul(
                            pr[:, c:c + 512], ones_c[:],
                            t[:, h * rpc + c:h * rpc + c + 512],
                            start=(k < 2), stop=(k >= nmm - 2),
                        )
                        k += 1

            nc.vector.tensor_copy(deg_l[:, :512], pr[:, :512])
            nc.scalar.copy(deg_l[:, 512:], pr[:, 512:])

        # ---- Phase B: AllGather raw degrees across the cores ----
        nc.gpsimd.dma_start(ag_in[:], deg_l[:])
        nc.gpsimd.collective_compute(
            "AllGather",
            mybir.AluOpType.bypass,
            replica_groups=[list(range(ncores))],
            ins=[ag_in.opt()],
            outs=[ag_out.opt()],
        )

        # local row-scale dinv_rep (overlaps the AllGather): broadcast the
        # degree row to all partitions, then rsqrt on the wide form — avoids
        # the very slow single-partition DVE reciprocal and any PE work.
        nc.gpsimd.partition_broadcast(deg_rep[:], deg_l[:])
        nc.scalar.sqrt(sq_rep[:], deg_rep[:])
        nc.vector.reciprocal(dinv_rep[:], sq_rep[:])

        # post-AG: ag_out viewed [64, 128] -> one PE transpose -> [128, 64]
        # degrees partition-major, then rsqrt: dinv_sb[p, j] = dinv[j*128 + p]
        nc.scalar.dma_start(ag_sb[:], ag_out[:].rearrange("r (g i) -> (r g) i", i=P))
        with tc.tile_pool(name="psum_t", bufs=1, space="PSUM") as psum_t:
            pt = psum_t.tile([P, jt], dt, name="pt")
            nc.tensor.transpose(pt[:], ag_sb[:], id64[:])
            nc.vector.reciprocal(rec_sb[:], pt[:])
        nc.scalar.sqrt(dinv_sb[:], rec_sb[:])

        # ---- Phase C: scale all support column blocks (DVE + Act in parallel)
        for j in range(jt):
            sl = slice(j * f, (j + 1) * f)
            if j % 2 == 0:
                nc.vector.tensor_scalar_mul(sup_bf[:, sl], sup_bf[:, sl],
                                            dinv_sb[:, j:j + 1])
            else:
                nc.scalar.mul(sup_bf[:, sl], sup_bf[:, sl], dinv_sb[:, j:j + 1])

        # ---- Phase D: out.T += S[j].T @ a_hat.T[j], back-to-back matmuls ----
        with tc.tile_pool(name="psum_o", bufs=1, space="PSUM") as psum_o:
            po = psum_o.tile([f, rpc], dt, name="po")
            alltiles = ([("full", i) for i in range(NFULL)]
                        + [("mini", i) for i in range(NMINI)])
            k = 0
            for it in alltiles:
                t = a_tiles[it]
                js = tile_js(it)
                for h, j in enumerate(js):
                    sl = slice(j * f, (j + 1) * f)
                    for c in (512, 0) if k >= 2 * jt - 2 else (0, 512):
                        nc.tensor.matmul(
                            po[:, c:c + 512], sup_bf[:, sl],
                            t[:, h * rpc + c:h * rpc + c + 512],
                            start=(k < 2), stop=(k >= 2 * jt - 2),
                        )
                        k += 1

            # ---- Phase E: out = dinv[i] * out, halves on two engines/queues
            nc.vector.tensor_mul(out_sb[:, 512:], po[:, 512:], dinv_rep[:, 512:])
            nc.scalar.dma_start(out_d[:, 512:], out_sb[:, 512:])
            nc.vector.tensor_mul(out_sb[:, :512], po[:, :512], dinv_rep[:, :512])
            nc.gpsimd.dma_start(out_d[:, :512], out_sb[:, :512])

    nc.compile()
    return nc


_module_cache = {}


def _get_module():
    if "nc" not in _module_cache:
        nc = build_module()
        from concourse.bass_interp import get_hw_module

        nc.m = get_hw_module(nc.m)
        _module_cache["nc"] = nc
    return _module_cache["nc"]


def make_in_maps(x, adjacency, W, b, n=N, f=F, ncores=NCORES):
    import ml_dtypes

    bfdt = ml_dtypes.bfloat16
    rpc = n // ncores
    x = np.asarray(x, dtype=np.float32)
    adjacency = np.asarray(adjacency, dtype=np.float32)
    W = np.asarray(W, dtype=np.float32)
    b = np.asarray(b, dtype=np.float32)
    xt = np.ascontiguousarray(x.T).astype(bfdt)
    wt = np.ascontiguousarray(W.T).astype(bfdt)
    bias = np.ascontiguousarray(b.reshape(1, f)).astype(np.float32)
    ones_r = np.ones((1, P), dtype=np.float32)
    ones_c = np.ones((P, 1), dtype=bfdt)
    id64 = np.eye(64, dtype=np.float32)
    in_maps = []
    for c in range(ncores):
        at = np.ascontiguousarray(adjacency[c * rpc:(c + 1) * rpc, :].T)
        # fold a_hat = A + I into the shard: global row c*rpc+i, column c*rpc+i
        at[c * rpc + np.arange(rpc), np.arange(rpc)] += 1.0
        # pre-tile into SBUF layout: full tiles get [P, FJ*rpc] with
        # att[base + p, h*rpc + i] = at[j0*128 + h*128 + p, i]; minis are 1:1.
        full = at[:NFULL * FJ * P].reshape(NFULL, FJ, P, rpc)
        full = full.transpose(0, 2, 1, 3).reshape(NFULL * FJ * P, rpc)
        att = np.concatenate([full, at[NFULL * FJ * P:]], axis=0)
        in_maps.append({
            "att": np.ascontiguousarray(att).astype(bfdt), "xt": xt, "wt": wt,
            "bias": bias, "ones_r": ones_r, "ones_c": ones_c, "id64": id64,
        })
    return in_maps


def kernel(x, adjacency, W, b):
    from concourse.bass_utils import run_bass_kernel_spmd

    nc = _get_module()
    in_maps = make_in_maps(x, adjacency, W, b)
    res = run_bass_kernel_spmd(nc, in_maps, core_ids=list(range(NCORES)))
    out = np.empty((N, F), dtype=np.float32)
    for c in range(NCORES):
        out[c * RPC:(c + 1) * RPC, :] = res.results[c]["out_t"].T
    return out


# revision 25
# speedup vs baseline: 1.0183x; 1.0183x over previous
"""GCN layer (normalized adjacency aggregation) on 8 Trainium2 NeuronCores.

Algorithm (row-sharded, bf16, fully SBUF-resident adjacency):
    a_hat = A + I  (identity folded into the shard on the host, then bf16)
    deg[i] = sum_j a_hat[i, j]              -> per-core PE ones-matmul over its rows
    AllGather raw degrees (4KB), rsqrt computed on both sides of the gather
    sup = x @ W.T + b  (bf16)               -> computed redundantly per core (tiny)
    S = dinv[:, None] * sup
    out[i, :] = dinv[i] * (a_hat[i, :] @ S) -> accumulating matmul over the row block

Performance structure:
  - the transposed adjacency shard is pre-tiled on the HOST into the exact SBUF
    tile layout (one contiguous run per partition), so tile DMAs need no
    software descriptor generation.  14 full tiles (4 j-blocks) + 8 mini tiles
    (1 j-block) ride three balanced DMA queues; x.T is split in halves across
    two queues; HBM is read exactly once and everything stays SBUF-resident.
  - the degree matmuls are emitted in estimated-arrival order of the tiles so
    the PE never head-of-line blocks on a late queue; mini tiles at the queue
    tails keep the post-stream backlog to ~2us.
  - support matmuls run early with a deep (6-buffer) PSUM pool so the trailing
    DVE bias-adds never stall the PE.
  - the AllGather moves raw degrees (PSUM copied out on DVE+Act halves); the
    gathered [64,128] view is transposed with one PE matmul into
    partition-major [128,64] and rsqrt'd there.
  - all 64 support-column scales run up front on DVE+Act, then the 128
    aggregation matmuls issue back-to-back; the final row-scale and output DMA
    are split in halves across two queues.
  - a tiny dummy collective keyed off a mid-stream tile DMA pre-wakes the CC
    cores so the real AllGather does not pay their ~11us wake latency.
"""

import numpy as np
from contextlib import ExitStack

N = 8192
F = 128
NCORES = 8
RPC = N // NCORES  # 1024 rows per core
P = 128            # partitions
JT = N // P        # 64 column tiles of the (transposed) block

NFULL = 12         # full tiles of FJ j-blocks each
FJ = 4
NMINI = JT - NFULL * FJ  # 16 mini tiles of 1 j-block

PREWAKE = True

# queue plans: lists of ("xt", half) | ("full", idx) | ("mini", idx) | ("consts",)
# gpsimd carries NO bulk DMA: its ring stays empty so the collective triggers
# (which only gpsimd can issue) are never blocked behind queued transfers.
QPLAN = {
    "sync":   [("xt", 0), ("full", 0), ("full", 2), ("full", 4), ("full", 6),
               ("full", 8), ("full", 10), ("mini", 0), ("mini", 2), ("mini", 4),
               ("mini", 6), ("mini", 8), ("mini", 10), ("mini", 12), ("mini", 14)],
    "gpsimd": [],
    "scalar": [("consts",), ("xt", 1), ("full", 1), ("full", 3), ("full", 5),
               ("full", 7), ("full", 9), ("full", 11), ("mini", 1), ("mini", 3),
               ("mini", 5), ("mini", 7), ("mini", 9), ("mini", 11),
               ("mini", 13), ("mini", 15)],
}
WAKE_AFTER = ("full", 5)  # dummy collective keyed off this tile (~33us arrival)


def _arrival_plan():
    """Estimated arrival times (us) per tile -> degree emission order."""
    rate = 0.15  # GB/s per queue in MB/us (two bulk queues sharing HBM)
    size = {"xt": 1.0, "full": 1.0, "mini": 0.25, "consts": 0.15}
    arrivals = {}
    for qname, items in QPLAN.items():
        t = 0.0
        for it in items:
            t += size[it[0]] / rate
            if it[0] in ("full", "mini"):
                arrivals[it] = t
    order = sorted(arrivals, key=lambda it: arrivals[it])
    return order


def build_module(n=N, f=F, ncores=NCORES, prewake=PREWAKE):
    """Build and compile the SPMD Bass module (same program on every core)."""
    from concourse import bass, bacc, tile

    mybir = bass.mybir
    dt = mybir.dt.float32
    bf = mybir.dt.bfloat16

    rpc = n // ncores
    jt = n // P

    nc = bacc.Bacc(
        "TRN2",
        target_bir_lowering=False,
        debug=False,
        enable_asserts=False,
        num_devices=ncores,
    )

    # att holds the pre-tiled shard: 14 full tiles then 8 mini tiles,
    # each [P, nj*rpc] with one contiguous run per partition.
    att_d = nc.dram_tensor("att", [(NFULL * FJ + NMINI) * P, rpc], bf,
                           kind="ExternalInput")
    xt_d = nc.dram_tensor("xt", [f, n], bf, kind="ExternalInput")
    wt_d = nc.dram_tensor("wt", [f, f], bf, kind="ExternalInput")
    b_d = nc.dram_tensor("bias", [1, f], dt, kind="ExternalInput")
    ones_r_d = nc.dram_tensor("ones_r", [1, P], dt, kind="ExternalInput")
    ones_c_d = nc.dram_tensor("ones_c", [P, 1], bf, kind="ExternalInput")
    id64_d = nc.dram_tensor("id64", [64, 64], dt, kind="ExternalInput")
    out_d = nc.dram_tensor("out_t", [f, rpc], dt, kind="ExternalOutput")

    def att_rows(it):
        """DRAM row range of tile `it` in att_d (viewed [P, nj*rpc])."""
        kind, idx = it
        if kind == "full":
            r0 = idx * FJ * P
            return r0, FJ
        r0 = (NFULL * FJ + idx) * P
        return r0, 1

    with tile.TileContext(nc) as tc, ExitStack() as ctx:
        cpool = ctx.enter_context(tc.tile_pool(name="const", bufs=1))
        wt_sb = cpool.tile([f, f], bf, name="wt_sb")
        bias_sb = cpool.tile([1, f], dt, name="bias_sb")
        ones_r = cpool.tile([1, P], dt, name="ones_r")
        ones_c = cpool.tile([P, 1], bf, name="ones_c")
        id64 = cpool.tile([64, 64], dt, name="id64")
        b_rep = cpool.tile([P, f], dt, name="b_rep")
        xt_sb = cpool.tile([f, n], bf, name="xt_sb")
        sup_bf = cpool.tile([P, jt * f], bf, name="sup_bf")
        deg_l = cpool.tile([1, rpc], dt, name="deg_l")
        deg_rep = cpool.tile([P, rpc], dt, name="deg_rep")
        sq_rep = cpool.tile([P, rpc], dt, name="sq_rep")
        rec_sb = cpool.tile([P, jt], dt, name="rec_sb")
        dinv_sb = cpool.tile([P, jt], dt, name="dinv_sb")
        dinv_rep = cpool.tile([P, rpc], dt, name="dinv_rep")
        ag_sb = cpool.tile([jt, P], dt, name="ag_sb")
        out_sb = cpool.tile([P, rpc], dt, name="out_sb")

        dram = ctx.enter_context(tc.tile_pool(name="dram", bufs=1, space="DRAM"))
        ag_in = dram.tile([1, rpc], dt, name="ag_in")
        ag_out = dram.tile([ncores, rpc], dt, name="ag_out",
                           addr_space="Shared" if ncores > 4 else "Local")
        if prewake:
            wk_in = dram.tile([1, 2], bf, name="wk_in")
            wk_out = dram.tile([ncores, 2], bf, name="wk_out",
                               addr_space="Shared" if ncores > 4 else "Local")

        afull = ctx.enter_context(tc.tile_pool(name="af", bufs=NFULL))
        amini = ctx.enter_context(tc.tile_pool(name="am", bufs=NMINI))
        a_tiles = {}
        engs = {"sync": nc.sync, "gpsimd": nc.gpsimd, "scalar": nc.scalar}
        for qname, items in QPLAN.items():
            eng = engs[qname]
            for it in items:
                if it[0] == "consts":
                    eng.dma_start(wt_sb[:], wt_d[:])
                    eng.dma_start(bias_sb[:], b_d[:])
                    eng.dma_start(ones_r[:], ones_r_d[:])
                    eng.dma_start(ones_c[:], ones_c_d[:])
                    eng.dma_start(id64[:], id64_d[:])
                elif it[0] == "xt":
                    h = it[1]
                    eng.dma_start(xt_sb[:, h * (n // 2):(h + 1) * (n // 2)],
                                  xt_d[:, h * (n // 2):(h + 1) * (n // 2)])
                else:
                    r0, nj = att_rows(it)
                    pool, tag = (afull, "af") if it[0] == "full" else (amini, "am")
                    t = pool.tile([P, nj * rpc], bf,
                                  name=f"a_{it[0]}{it[1]}", tag=tag)
                    eng.dma_start(t[:], att_d[r0:r0 + nj * P, :].rearrange(
                        "(p h) i -> p (h i)", p=P))
                    a_tiles[it] = t
                if prewake and it == WAKE_AFTER:
                    eng.dma_start(wk_in[:], a_tiles[it][0:1, 0:2])

        def tile_js(it):
            """Global j indices covered by tile `it`."""
            kind, idx = it
            if kind == "full":
                return [idx * FJ + h for h in range(FJ)]
            return [NFULL * FJ + idx]

        # ---- Phase A: support = x @ W.T + b, and degree row sums ----
        with (
            tc.tile_pool(name="psum_s", bufs=5, space="PSUM") as psum_s,
            tc.tile_pool(name="psum_b", bufs=1, space="PSUM") as psum_b,
            tc.tile_pool(name="psum_r", bufs=1, space="PSUM") as psum_r,
        ):
            # bias broadcast via outer product: ones_r.T @ bias -> [P, f]
            pb = psum_b.tile([P, f], dt, name="pb")
            nc.tensor.matmul(pb[:], ones_r[:], bias_sb[:], start=True, stop=True)
            nc.vector.tensor_copy(b_rep[:], pb[:])

            for j in range(jt):
                ps = psum_s.tile([P, f], dt, name=f"ps{j}", tag="ps")
                nc.tensor.matmul(ps[:], xt_sb[:, j * f:(j + 1) * f], wt_sb[:],
                                 start=True, stop=True)
                nc.vector.tensor_add(sup_bf[:, j * f:(j + 1) * f], ps[:], b_rep[:])

            if prewake:
                nc.gpsimd.collective_compute(
                    "AllGather",
                    mybir.AluOpType.bypass,
                    replica_groups=[list(range(ncores))],
                    ins=[wk_in.opt()],
                    outs=[wk_out.opt()],
                )

            # degree: accumulate ones_c.T @ a_tile into [1, rpc], tiles in
            # estimated-arrival order (accumulation is commutative)
            order = _arrival_plan()
            nmm = sum(len(tile_js(it)) for it in order) * 2
            pr = psum_r.tile([1, rpc], dt, name="pr")
            k = 0
            for it in order:
                t = a_tiles[it]
                for h in range(len(tile_js(it))):
                    for c in range(0, rpc, 512):
                        nc.tensor.matmul(
                            pr[:, c:c + 512], ones_c[:],
                            t[:, h * rpc + c:h * rpc + c + 512],
                            start=(k < 2), stop=(k >= nmm - 2),
                        )
                        k += 1

            nc.vector.tensor_copy(deg_l[:, :512], pr[:, :512])
            nc.scalar.copy(deg_l[:, 512:], pr[:, 512:])

        # ---- Phase B: AllGather raw degrees across the cores ----
        nc.gpsimd.dma_start(ag_in[:], deg_l[:])
        nc.gpsimd.collective_compute(
            "AllGather",
            mybir.AluOpType.bypass,
            replica_groups=[list(range(ncores))],
            ins=[ag_in.opt()],
            outs=[ag_out.opt()],
        )

        # local row-scale dinv_rep (overlaps the AllGather): broadcast the
        # degree row to all partitions, then rsqrt on the wide form — avoids
        # the very slow single-partition DVE reciprocal and any PE work.
        nc.gpsimd.partition_broadcast(deg_rep[:], deg_l[:])
        nc.scalar.sqrt(sq_rep[:], deg_rep[:])
        nc.vector.reciprocal(dinv_rep[:], sq_rep[:])

        # post-AG: ag_out viewed [64, 128] -> one PE transpose -> [128, 64]
        # degrees partition-major, then rsqrt: dinv_sb[p, j] = dinv[j*128 + p]
        nc.scalar.dma_start(ag_sb[:], ag_out[:].rearrange("r (g i) -> (r g) i", i=P))
        with tc.tile_pool(name="psum_t", bufs=1, space="PSUM") as psum_t:
            pt = psum_t.tile([P, jt], dt, name="pt")
            # two halves so the first scales (and phase D) start earlier
            for g0 in (0, jt // 2):
                gs = slice(g0, g0 + jt // 2)
                nc.tensor.transpose(pt[:, gs], ag_sb[gs, :], id64[gs, gs])
                nc.vector.reciprocal(rec_sb[:, gs], pt[:, gs])
                nc.scalar.sqrt(dinv_sb[:, gs], rec_sb[:, gs])

                # ---- Phase C: scale support column blocks (DVE + Act) ----
                for j in range(g0, g0 + jt // 2):
                    sl = slice(j * f, (j + 1) * f)
                    if j % 2 == 0:
                        nc.vector.tensor_scalar_mul(sup_bf[:, sl], sup_bf[:, sl],
                                                    dinv_sb[:, j:j + 1])
                    else:
                        nc.scalar.mul(sup_bf[:, sl], sup_bf[:, sl],
                                      dinv_sb[:, j:j + 1])

        # ---- Phase D: out.T += S[j].T @ a_hat.T[j], back-to-back matmuls ----
        with tc.tile_pool(name="psum_o", bufs=1, space="PSUM") as psum_o:
            po = psum_o.tile([f, rpc], dt, name="po")
            alltiles = ([("full", i) for i in range(NFULL)]
                        + [("mini", i) for i in range(NMINI)])
            k = 0
            for it in alltiles:
                t = a_tiles[it]
                js = tile_js(it)
                for h, j in enumerate(js):
                    sl = slice(j * f, (j + 1) * f)
                    for c in (512, 0) if k >= 2 * jt - 2 else (0, 512):
                        nc.tensor.matmul(
                            po[:, c:c + 512], sup_bf[:, sl],
                            t[:, h * rpc + c:h * rpc + c + 512],
                            start=(k < 2), stop=(k >= 2 * jt - 2),
                        )
                        k += 1

            # ---- Phase E: out = dinv[i] * out, halves on two engines/queues
            nc.vector.tensor_mul(out_sb[:, 512:], po[:, 512:], dinv_rep[:, 512:])
            nc.scalar.dma_start(out_d[:, 512:], out_sb[:, 512:])
            nc.vector.tensor_mul(out_sb[:, :512], po[:, :512], dinv_rep[:, :512])
            nc.gpsimd.dma_start(out_d[:, :512], out_sb[:, :512])

    nc.compile()
    return nc


_module_cache = {}


def _get_module():
    if "nc" not in _module_cache:
        nc = build_module()
        from concourse.bass_interp import get_hw_module

        nc.m = get_hw_module(nc.m)
        _module_cache["nc"] = nc
    return _module_cache["nc"]


def make_in_maps(x, adjacency, W, b, n=N, f=F, ncores=NCORES):
    import ml_dtypes

    bfdt = ml_dtypes.bfloat16
    rpc = n // ncores
    x = np.asarray(x, dtype=np.float32)
    adjacency = np.asarray(adjacency, dtype=np.float32)
    W = np.asarray(W, dtype=np.float32)
    b = np.asarray(b, dtype=np.float32)
    xt = np.ascontiguousarray(x.T).astype(bfdt)
    wt = np.ascontiguousarray(W.T).astype(bfdt)
    bias = np.ascontiguousarray(b.reshape(1, f)).astype(np.float32)
    ones_r = np.ones((1, P), dtype=np.float32)
    ones_c = np.ones((P, 1), dtype=bfdt)
    id64 = np.eye(64, dtype=np.float32)
    in_maps = []
    for c in range(ncores):
        at = np.ascontiguousarray(adjacency[c * rpc:(c + 1) * rpc, :].T)
        # fold a_hat = A + I into the shard: global row c*rpc+i, column c*rpc+i
        at[c * rpc + np.arange(rpc), np.arange(rpc)] += 1.0
        # pre-tile into SBUF layout: full tiles get [P, FJ*rpc] with
        # att[base + p, h*rpc + i] = at[j0*128 + h*128 + p, i]; minis are 1:1.
        full = at[:NFULL * FJ * P].reshape(NFULL, FJ, P, rpc)
        full = full.transpose(0, 2, 1, 3).reshape(NFULL * FJ * P, rpc)
        att = np.concatenate([full, at[NFULL * FJ * P:]], axis=0)
        in_maps.append({
            "att": np.ascontiguousarray(att).astype(bfdt), "xt": xt, "wt": wt,
            "bias": bias, "ones_r": ones_r, "ones_c": ones_c, "id64": id64,
        })
    return in_maps


def kernel(x, adjacency, W, b):
    from concourse.bass_utils import run_bass_kernel_spmd

    nc = _get_module()
    in_maps = make_in_maps(x, adjacency, W, b)
    res = run_bass_kernel_spmd(nc, in_maps, core_ids=list(range(NCORES)))
    out = np.empty((N, F), dtype=np.float32)
    for c in range(NCORES):
        out[c * RPC:(c + 1) * RPC, :] = res.results[c]["out_t"].T
    return out
